# revision 52
# baseline (speedup 1.0000x reference)
"""DeepseekV2 MLA prefill kernel for 8 Trainium2 NeuronCores.

Sharding:
  Launch A: sequence-parallel fused qkv_a projection + RMSNorms + k_pe rope.
            Each core: T/8 = 256 tokens. Host pre-transposes hidden_states
            so no on-device transposes are needed. LN weights are folded
            into w_qb/w_kvb on the host (exact: per-feature scale commutes
            into the next matmul's rows), so A only applies the rsqrt
            row-scale.
  Host:     gather token slices -> full feature-major activations.
  Launch B: tensor-parallel over heads (2 heads/core): q_b / kv_b
            projections (ColumnParallel), q rope, causal attention with
            TRANSPOSED scores (stationary = K tile, moving = Q chunk) so
            probabilities come out key-major and feed PV directly (no PE
            transposes, no PSUM->SBUF prob copies). Softmax normalization
            is deferred: unnormalized exp -> PV + rowsums (ones-matmul),
            attnT scaled once per 512-query chunk. o_proj RowParallel.
  Host:     sum the 8 partial outputs (the RowParallel all-reduce).
"""

import numpy as np
import concourse.bacc as bacc
import concourse.tile as tile
from concourse import mybir
from concourse import bass_utils

F32 = mybir.dt.float32
F32R = mybir.dt.float32r
F16 = mybir.dt.float16
AF = mybir.ActivationFunctionType
AX = mybir.AxisListType

NCORES = 8
T, HID, H = 2048, 5120, 16
NOPE, ROPE, VDIM = 128, 64, 128
QLORA, KVLORA = 1536, 512
FUSED = QLORA + KVLORA + ROPE  # 2112
TS = T // NCORES               # 256 tokens/core in launch A
HPC = H // NCORES              # 2 heads/core in launch B
EPS = 1e-6
THETA = 10000.0
SCALING = float((NOPE + ROPE) ** -0.5)
NEG = -1.0e30
KT = HID // 128                # 40
MT = (FUSED + 127) // 128      # 17 (last tile 64 rows)
TT = T // 128                  # 16
NCH = T // 512                 # 4


def _build_a():
    nc = bacc.Bacc("TRN2", target_bir_lowering=False, debug=False,
                   num_devices=NCORES)
    hidT_s = nc.dram_tensor("hidT_s", [HID, TS], F16,
                            kind="ExternalInput").ap()
    w_fused = nc.dram_tensor("w_fused", [HID, FUSED], F16,
                             kind="ExternalInput").ap()
    cos_a = nc.dram_tensor("cos_a", [ROPE, TS], F16,
                           kind="ExternalInput").ap()
    sin_a = nc.dram_tensor("sin_a", [ROPE, TS], F16,
                           kind="ExternalInput").ap()
    swap64t = nc.dram_tensor("swap64t", [ROPE, ROPE], F16,
                             kind="ExternalInput").ap()
    q_aT_s = nc.dram_tensor("q_aT_s", [QLORA, TS], F16,
                            kind="ExternalOutput").ap()
    kv_aT_s = nc.dram_tensor("kv_aT_s", [KVLORA, TS], F16,
                             kind="ExternalOutput").ap()
    k_peT_s = nc.dram_tensor("k_peT_s", [ROPE, TS], F16,
                             kind="ExternalOutput").ap()

    with tile.TileContext(nc) as tc:
        with tc.tile_pool(name="consts", bufs=1) as consts, \
             tc.tile_pool(name="hidT_pool", bufs=1) as hidT_pool, \
             tc.tile_pool(name="qkv_pool", bufs=1) as qkv_pool, \
             tc.tile_pool(name="small", bufs=1) as small:
            ones_f32 = consts.tile([128, 1], F32)
            nc.vector.memset(ones_f32, 1.0)
            ones_col = consts.tile([128, 1], F32R)
            nc.vector.tensor_copy(ones_col, ones_f32)
            ones_row_f32 = consts.tile([1, 128], F32)
            nc.vector.memset(ones_row_f32, 1.0)
            ones_row = consts.tile([1, 128], F32R)
            nc.vector.tensor_copy(ones_row, ones_row_f32)
            swap_sb = consts.tile([ROPE, ROPE], F16)
            cos_sb = consts.tile([ROPE, TS], F16)
            sin_sb = consts.tile([ROPE, TS], F16)
            eps_sb = small.tile([1, 1], F32)
            nc.vector.memset(eps_sb, EPS)

            qkvT = qkv_pool.tile([128, 16, TS], F16)    # raw q|kv staging
            qkv16 = qkv_pool.tile([128, 16, TS], F16)   # normalized
            # all chunks >=256 cols: narrower slices fall below the 512B
            # contiguous-row threshold and transfer at half DMA bandwidth
            chunks = [[0, 1], [2, 3], [4, 5], [6, 7], [8, 9], [10, 11],
                      [12, 13], [14, 15, 16]]
            with tc.tile_pool(name="wpool", bufs=3) as wp, \
                 tc.tile_pool(name="mpsum", bufs=3, space="PSUM") as mp, \
                 tc.tile_pool(name="ropeps", bufs=1, space="PSUM") as rps, \
                 tc.tile_pool(name="sqpool", bufs=3) as sqp, \
                 tc.tile_pool(name="kpe_p", bufs=1) as kpe_p, \
                 tc.tile_pool(name="sumps", bufs=1, space="PSUM") as sums_pool:
                sq_ps_q = sums_pool.tile([1, TS], F32, tag="sq_q")
                sq_ps_kv = sums_pool.tile([1, TS], F32, tag="sq_kv")
                bq_ps = sums_pool.tile([128, TS], F32, tag="bq")
                bkv_ps = sums_pool.tile([128, TS], F32, tag="bkv")

                def chunk_dma(ms):
                    c0 = ms[0] * 128
                    ccols = min(FUSED, (ms[-1] + 1) * 128) - c0
                    wt = wp.tile([128, KT, 320], F16, tag="wt")
                    nc.sync.dma_start(
                        out=wt[:, :, :ccols],
                        in_=w_fused[:, c0:c0 + ccols].rearrange(
                            "(kt p) m -> p kt m", p=128))
                    return wt

                # first weight chunk, then hidT in halves; consts deferred
                wts = {0: chunk_dma(chunks[0])}
                hidT = hidT_pool.tile([128, KT, TS], F16)
                for half in range(2):
                    nc.sync.dma_start(
                        out=hidT[:, half * 20:(half + 1) * 20, :],
                        in_=hidT_s[half * 20 * 128:(half + 1) * 20 * 128, :]
                        .rearrange("(kt p) t -> p kt t", p=128))
                nc.sync.dma_start(out=swap_sb, in_=swap64t)
                nc.sync.dma_start(out=cos_sb, in_=cos_a)
                nc.sync.dma_start(out=sin_sb, in_=sin_a)

                rq_r = small.tile([1, TS], F32R, tag="rq_r")
                rkv_r = small.tile([1, TS], F32R, tag="rkv_r")

                def flush_sq(pend):
                    # sumsq matmul lagged one tile so PE never waits on the
                    # Act square of the tile it just produced
                    m, sq = pend
                    tgt = sq_ps_q if m < 12 else sq_ps_kv
                    nc.tensor.matmul(tgt, ones_col, sq,
                                     start=(m in (0, 12)),
                                     stop=(m in (11, 15)),
                                     skip_group_check=True)
                    if m == 11:
                        rq = small.tile([1, TS], F32, tag="rq")
                        nc.scalar.activation(rq, sq_ps_q, func=AF.Sqrt,
                                             scale=1.0 / QLORA, bias=eps_sb)
                        nc.vector.reciprocal(rq, rq)
                        nc.vector.tensor_copy(rq_r, rq)
                    if m == 15:
                        rkv = small.tile([1, TS], F32, tag="rkv")
                        nc.scalar.activation(rkv, sq_ps_kv, func=AF.Sqrt,
                                             scale=1.0 / KVLORA, bias=eps_sb)
                        nc.vector.reciprocal(rkv, rkv)
                        nc.vector.tensor_copy(rkv_r, rkv)

                pend_sq = None
                for ci, ms in enumerate(chunks):
                    wt = wts.pop(ci)
                    if ci + 1 < len(chunks):
                        wts[ci + 1] = chunk_dma(chunks[ci + 1])
                    for mi, m in enumerate(ms):
                        mm = min(128, FUSED - m * 128)
                        ps = mp.tile([128, TS], F32, tag="ps")
                        for k in range(KT):
                            nc.tensor.matmul(
                                ps[:mm],
                                wt[:, k, mi * 128:mi * 128 + mm],
                                hidT[:, k, :],
                                start=(k == 0), stop=(k == KT - 1))
                            if k == 9 and m == 16 and pend_sq is not None:
                                # flush kv sumsq early inside the last
                                # tile so its rsqrt chain finishes before
                                # the broadcast matmul needs it
                                flush_sq(pend_sq)
                                pend_sq = None
                        if pend_sq is not None:
                            flush_sq(pend_sq)
                            pend_sq = None
                        if m == 16:
                            # k_pe: rope directly (no norm)
                            kpe_sb = kpe_p.tile([ROPE, TS], F16)
                            nc.vector.tensor_copy(kpe_sb, ps[:ROPE])
                            ps2 = rps.tile([ROPE, TS], F32, tag="swk")
                            nc.tensor.matmul(ps2, swap_sb, kpe_sb,
                                             start=True, stop=True)
                            kro = kpe_p.tile([ROPE, TS], F16)
                            nc.vector.tensor_mul(kro, kpe_sb, cos_sb)
                            nc.vector.tensor_mul(ps2, ps2, sin_sb)
                            nc.vector.tensor_add(kro, kro, ps2)
                            nc.sync.dma_start(out=k_peT_s, in_=kro)
                            continue
                        nc.scalar.copy(qkvT[:, m, :], ps)
                        sq = sqp.tile([128, TS], F32R, tag="sq")
                        nc.scalar.square(sq, ps)
                        pend_sq = (m, sq)
                        if m == 14:
                            nc.tensor.matmul(bq_ps, ones_row, rq_r,
                                             start=True, stop=True)
                            for mq in range(12):
                                nc.vector.tensor_mul(qkv16[:, mq, :],
                                                     qkvT[:, mq, :], bq_ps)
                            nc.sync.dma_start(
                                out=q_aT_s.rearrange("(mt p) t -> p mt t",
                                                     p=128),
                                in_=qkv16[:, 0:12, :])
                # kv norm finalize: broadcast matmul issued after the m=16
                # matmuls so the rkv chain is long done
                nc.tensor.matmul(bkv_ps, ones_row, rkv_r,
                                 start=True, stop=True)
                for mq in range(12, 16):
                    nc.vector.tensor_mul(qkv16[:, mq, :],
                                         qkvT[:, mq, :], bkv_ps)
                nc.sync.dma_start(
                    out=kv_aT_s.rearrange("(mt p) t -> p mt t", p=128),
                    in_=qkv16[:, 12:16, :])
    nc.compile()
    return nc


def _build_b():
    nc = bacc.Bacc("TRN2", target_bir_lowering=False, debug=False,
                   num_devices=NCORES)
    q_aT = nc.dram_tensor("q_aT", [QLORA, T], F16, kind="ExternalInput").ap()
    kv_aT = nc.dram_tensor("kv_aT", [KVLORA, T], F16,
                           kind="ExternalInput").ap()
    k_peT = nc.dram_tensor("k_peT", [ROPE, T], F16,
                           kind="ExternalInput").ap()   # already roped
    w_qb_s = nc.dram_tensor("w_qb_s", [QLORA, HPC * (NOPE + ROPE)], F16,
                            kind="ExternalInput").ap()
    # w_kvb_s host layout: cols = [h0 nope, h1 nope, h0 v, h1 v]
    w_kvb_s = nc.dram_tensor("w_kvb_s", [KVLORA, HPC * (NOPE + VDIM)], F16,
                             kind="ExternalInput").ap()
    w_o_s = nc.dram_tensor("w_o_s", [HPC * VDIM, HID], F16,
                           kind="ExternalInput").ap()
    cos2 = nc.dram_tensor("cos2", [128, T], F16, kind="ExternalInput").ap()
    sin2 = nc.dram_tensor("sin2", [128, T], F16, kind="ExternalInput").ap()
    swap2t = nc.dram_tensor("swap2t", [128, 128], F16,
                            kind="ExternalInput").ap()
    # transposed-scores causal mask, pre-transposed for the stationary slot
    # of a mask-add matmul (mask = diagt_t.T @ I)
    diagt_t = nc.dram_tensor("diagt_t", [128, 128], F16,
                             kind="ExternalInput").ap()
    ident = nc.dram_tensor("ident", [128, 128], F16,
                           kind="ExternalInput").ap()
    o_part = nc.dram_tensor("o_part", [T, HID], F16,
                            kind="ExternalOutput").ap()

    with tile.TileContext(nc) as tc:
        with tc.tile_pool(name="consts", bufs=1) as consts, \
             tc.tile_pool(name="attn_out", bufs=1) as attn_out:
            # tiles allocated here; DMAs issued later (just-in-time order)
            swap_sb = consts.tile([128, 128], F16)
            diagt_sb = consts.tile([128, 128], F16)
            ident_sb = consts.tile([128, 128], F16)
            ones_f32 = consts.tile([128, 1], F32)
            nc.vector.memset(ones_f32, 1.0)
            ones_col = consts.tile([128, 1], F16)
            nc.vector.tensor_copy(ones_col, ones_f32)
            ones_row_f32 = consts.tile([1, 128], F32)
            nc.vector.memset(ones_row_f32, 1.0)
            ones_row = consts.tile([1, 128], F16)
            nc.vector.tensor_copy(ones_row, ones_row_f32)
            attnT = attn_out.tile([128, HPC, T], F16)

            with tc.tile_pool(name="qk", bufs=1) as qk:
                qn_t = qk.tile([128, HPC, T], F16)    # q nope, per head
                qpe_ro = qk.tile([128, T], F16)
                qpe_ro_h1 = qk.tile([64, T], F16)     # h1 rows rebased to p0
                kn_t = qk.tile([128, HPC, T], F16)    # k nope, per head
                kpe_sb = qk.tile([ROPE, T], F16)      # roped in launch A
                vt = qk.tile([128, TT, HPC * VDIM], F16)  # v token-major

                # ---- kv_b projection -> feature-major k_nope + token-major v
                with tc.tile_pool(name="kva_p", bufs=4) as kva_p, \
                     tc.tile_pool(name="wkvb_p", bufs=1) as wkvb_p, \
                     tc.tile_pool(name="kvpsum", bufs=3, space="PSUM") as kvps:
                    # DMA issue order is delivery order: interleave the
                    # q-phase inputs so the q projection isn't starved
                    wkvb = wkvb_p.tile([128, 4, 512], F16)
                    kva_tiles = []
                    for _ in range(NCH):
                        kva_n = kva_p.tile([128, 4, 512], F16, tag="kva")
                        kva_tiles.append(kva_n)
                    # interleave weight column-pieces with the first chunk's
                    # k-slices in consumption order so matmuls start ASAP
                    nc.sync.dma_start(
                        out=wkvb[:, :, 0:128],
                        in_=w_kvb_s[:, 0:128].rearrange(
                            "(kt p) m -> p kt m", p=128))
                    for k in range(4):
                        nc.sync.dma_start(
                            out=kva_tiles[0][:, k, :],
                            in_=kv_aT[k * 128:(k + 1) * 128, 0:512])
                        if k == 1:
                            nc.sync.dma_start(
                                out=wkvb[:, :, 128:256],
                                in_=w_kvb_s[:, 128:256].rearrange(
                                    "(kt p) m -> p kt m", p=128))
                    nc.sync.dma_start(
                        out=wkvb[:, :, 256:512],
                        in_=w_kvb_s[:, 256:512].rearrange(
                            "(kt p) m -> p kt m", p=128))
                    for n in range(1, NCH):
                        ncol = slice(n * 512, (n + 1) * 512)
                        nc.sync.dma_start(
                            out=kva_tiles[n],
                            in_=kv_aT[:, ncol].rearrange(
                                "(kt p) t -> p kt t", p=128))
                    wqb_pre = qk.tile([128, 12, 384], F16)
                    nc.sync.dma_start(
                        out=wqb_pre,
                        in_=w_qb_s.rearrange("(kt p) m -> p kt m", p=128))
                    qa0_pre = qk.tile([128, 12, 512], F16)
                    nc.sync.dma_start(
                        out=qa0_pre,
                        in_=q_aT[:, 0:512].rearrange(
                            "(kt p) t -> p kt t", p=128))
                    for n in range(NCH):
                        ncol = slice(n * 512, (n + 1) * 512)
                        kva_n = kva_tiles[n]
                        for h in range(HPC):
                            ps = kvps.tile([128, 512], F32, tag="knmm")
                            for k in range(4):
                                nc.tensor.matmul(
                                    ps, wkvb[:, k, h * 128:(h + 1) * 128],
                                    kva_n[:, k, :],
                                    start=(k == 0), stop=(k == 3))
                            nc.scalar.copy(kn_t[:, h, ncol], ps)
                        for stl in range(4):
                            st = n * 4 + stl
                            ps = kvps.tile([128, 256], F32, tag="vmm")
                            for k in range(4):
                                nc.tensor.matmul(
                                    ps,
                                    kva_n[:, k, stl * 128:(stl + 1) * 128],
                                    wkvb[:, k, 256:512],
                                    start=(k == 0), stop=(k == 3))
                            nc.scalar.copy(vt[:, st, :], ps)

                # ---- q_b projection -> feature-major q, with rope
                #      interleaved per 512-token chunk
                with tc.tile_pool(name="qa_p", bufs=2) as qa_p, \
                     tc.tile_pool(name="ropec", bufs=1) as ropec, \
                     tc.tile_pool(name="qpe_p", bufs=1) as qpe_p, \
                     tc.tile_pool(name="rps", bufs=2, space="PSUM") as rps, \
                     tc.tile_pool(name="qpsum", bufs=3, space="PSUM") as qps:
                    cos_sb = ropec.tile([128, T], F16)
                    sin_sb = ropec.tile([128, T], F16)
                    qpe = qpe_p.tile([128, T], F16)   # q pe stacked h0|h1
                    wqb = wqb_pre
                    def do_rope(ncol):
                        # rope for one chunk (both heads stacked)
                        psr = rps.tile([128, 512], F32, tag="swq")
                        nc.tensor.matmul(psr, swap_sb, qpe[:, ncol],
                                         start=True, stop=True)
                        nc.vector.tensor_mul(qpe_ro[:, ncol], qpe[:, ncol],
                                             cos_sb[:, ncol])
                        nc.vector.tensor_mul(psr, psr, sin_sb[:, ncol])
                        nc.vector.tensor_add(qpe_ro[:, ncol], qpe_ro[:, ncol],
                                             psr)
                        nc.vector.tensor_copy(qpe_ro_h1[:, ncol],
                                              qpe_ro[64:128, ncol])

                    pend_rope = None
                    for n in range(NCH):
                        ncol = slice(n * 512, (n + 1) * 512)
                        if n == 0:
                            qa_n = qa0_pre
                        else:
                            qa_n = qa_p.tile([128, 12, 512], F16, tag="qa")
                            nc.sync.dma_start(
                                out=qa_n,
                                in_=q_aT[:, ncol].rearrange(
                                    "(kt p) t -> p kt t", p=128))
                        if n == 1:
                            nc.sync.dma_start(out=cos_sb, in_=cos2)
                            nc.sync.dma_start(out=sin_sb, in_=sin2)
                            nc.sync.dma_start(out=swap_sb, in_=swap2t)
                        if n == 3:
                            nc.sync.dma_start(out=diagt_sb, in_=diagt_t)
                            nc.sync.dma_start(out=ident_sb, in_=ident)
                            nc.sync.dma_start(out=kpe_sb, in_=k_peT)
                        # last chunk: qpe tiles (m=1,2) first so its rope
                        # never blocks the attention matmuls behind it in
                        # the in-order PE queue
                        for mi, m in enumerate([0, 1, 2] if n < 3
                                               else [1, 2, 0]):
                            if mi == 1 and pend_rope is not None:
                                # flush BEFORE this tile's qpe copies are
                                # emitted: tile-granular dep tracking would
                                # otherwise chain the swap matmul onto them
                                do_rope(pend_rope)
                                pend_rope = None
                            ps = qps.tile([128, 512], F32, tag="qmm")
                            for k in range(12):
                                nc.tensor.matmul(
                                    ps, wqb[:, k, m * 128:(m + 1) * 128],
                                    qa_n[:, k, :],
                                    start=(k == 0), stop=(k == 11))
                            if m == 0:
                                nc.scalar.copy(qn_t[:, 0, ncol], ps)
                            elif m == 1:
                                nc.scalar.copy(qpe[0:64, ncol], ps[0:64])
                                nc.scalar.copy(qn_t[0:64, 1, ncol],
                                               ps[64:128])
                            else:
                                nc.scalar.copy(qn_t[64:128, 1, ncol],
                                               ps[0:64])
                                nc.scalar.copy(qpe[64:128, ncol],
                                               ps[64:128])
                        if n < 3:
                            pend_rope = ncol
                        else:
                            do_rope(ncol)

                # ---- causal attention, transposed scores: for each 512-query
                #      chunk, loop key tiles; probs come out key-major and
                #      feed PV directly. Normalization deferred to attnT.
                with tc.tile_pool(name="ep", bufs=4) as ep, \
                     tc.tile_pool(name="smp", bufs=4) as smp, \
                     tc.tile_pool(name="bcsb_p", bufs=2) as bcsb_p, \
                     tc.tile_pool(name="osb_p", bufs=2) as osb_p, \
                     tc.tile_pool(name="wo_p", bufs=1) as wo_p, \
                     tc.tile_pool(name="scps", bufs=2, space="PSUM") as scps, \
                     tc.tile_pool(name="rsps", bufs=1, space="PSUM") as rsps, \
                     tc.tile_pool(name="pvps", bufs=2, space="PSUM") as pvps, \
                     tc.tile_pool(name="ops", bufs=3, space="PSUM") as ops:
                    wo = wo_p.tile([128, HPC, HID], F16)
                    nc.sync.dma_start(
                        out=wo, in_=w_o_s.rearrange("(kt p) m -> p kt m",
                                                    p=128))
                    def do_oproj(cq):
                        # o_proj for one chunk's 4 token tiles (RowParallel)
                        for tl in range(4):
                            tt = cq * 4 + tl
                            tcol = slice(tt * 128, (tt + 1) * 128)
                            osb = osb_p.tile([128, HID], F16, tag="osb")
                            for nch in range(HID // 512):
                                ps = ops.tile([128, 512], F32, tag="op")
                                for h in range(HPC):
                                    nc.tensor.matmul(
                                        ps, attnT[:, h, tcol],
                                        wo[:, h, nch * 512:(nch + 1) * 512],
                                        start=(h == 0), stop=(h == HPC - 1))
                                # split so each engine stays near the PE
                                # pace and the PSUM ring doesn't stall
                                # whole-tile copies alternating engines:
                                # with ops bufs=3 latency is hidden and one
                                # copy per nch halves the fixed access cost
                                dst = osb[:, nch * 512:(nch + 1) * 512]
                                if nch % 2 == 0:
                                    nc.vector.tensor_copy(dst, ps)
                                else:
                                    nc.scalar.copy(dst, ps)
                                last_tt = tt == TT - 1
                                if not last_tt and nch == 4:
                                    nc.sync.dma_start(
                                        out=o_part[tt * 128:(tt + 1) * 128,
                                                   0:2560],
                                        in_=osb[:, 0:2560])
                                if last_tt and (nch % 2 == 1 or nch == 8):
                                    # stream the final tile; last two writes
                                    # are single 512-col chunks so the tail
                                    # is as short as possible
                                    c0 = (nch - 1) * 512 if nch < 8 else \
                                        nch * 512
                                    cw = 1024 if nch < 8 else 512
                                    nc.sync.dma_start(
                                        out=o_part[tt * 128:(tt + 1) * 128,
                                                   c0:c0 + cw],
                                        in_=osb[:, c0:c0 + cw])
                            if tt != TT - 1:
                                nc.sync.dma_start(
                                    out=o_part[tt * 128:(tt + 1) * 128,
                                               2560:],
                                    in_=osb[:, 2560:])

                    def rs_chain(rs_ps):
                        # DVE-only: drain the rowsum PSUM bank right away
                        rt = smp.tile([1, 512], F32, tag="rt")
                        nc.vector.reciprocal(rt, rs_ps)
                        rt16 = smp.tile([1, 512], F16, tag="rt16")
                        nc.vector.tensor_copy(rt16, rt)
                        return rt16

                    def finalize_head(cq, h, rt16, pv_ps):
                        # deferred softmax normalization of the PV result
                        qchunk = slice(cq * 512, (cq + 1) * 512)
                        bc_ps = scps.tile([128, 512], F32, tag="sc")
                        nc.tensor.matmul(bc_ps, ones_row, rt16,
                                         start=True, stop=True)
                        bc_sb = bcsb_p.tile([128, 512], F32, tag="bcsb")
                        nc.scalar.copy(bc_sb, bc_ps)
                        nc.vector.tensor_mul(attnT[:, h, qchunk],
                                             pv_ps, bc_sb)

                    def flush_pend(pend):
                        # 1-tile lag: PE consumes exp output a tile late so
                        # it never waits on Act; the lag crosses the head
                        # boundary so even the last tile's consume is hidden
                        ph, pst, pet, pnq, poff, prs, ppv, plast = pend
                        nc.tensor.matmul(
                            prs[:, poff:512], ones_col, pet[:, :pnq],
                            start=(pst == 0), stop=plast,
                            skip_group_check=True)
                        nc.tensor.matmul(
                            ppv[:, poff:512],
                            vt[:, pst, ph * 128:(ph + 1) * 128],
                            pet[:, :pnq],
                            start=(pst == 0), stop=plast,
                            skip_group_check=True)
                        if plast:
                            return rs_chain(prs), ppv
                        return None

                    for cq in range(NCH):
                        nst = 4 * (cq + 1)
                        head_fin = {}
                        pend = None
                        for h in range(HPC):
                            rs_ps = rsps.tile([1, 512], F32, tag="rs")
                            pv_ps = pvps.tile([128, 512], F32, tag="pv")
                            for st in range(nst):
                                stcol = slice(st * 128, (st + 1) * 128)
                                off = max(0, (st - 4 * cq) * 128)
                                nq = 512 - off
                                qcol = slice(cq * 512 + off, (cq + 1) * 512)
                                ps = scps.tile([128, 512], F32, tag="sc")
                                nc.tensor.matmul(ps[:, :nq],
                                                 kn_t[:, h, stcol],
                                                 qn_t[:, h, qcol],
                                                 start=True, stop=False)
                                qpe_l = (qpe_ro[0:64, qcol] if h == 0
                                         else qpe_ro_h1[:, qcol])
                                diag = st >= 4 * cq
                                nc.tensor.matmul(ps[:, :nq],
                                                 kpe_sb[:, stcol], qpe_l,
                                                 start=False, stop=not diag)
                                if diag:
                                    # causal mask via PE: ps[:,0:128] += diagT
                                    nc.tensor.matmul(
                                        ps[:, 0:128], diagt_sb, ident_sb,
                                        start=False, stop=True,
                                        skip_group_check=True)
                                et = ep.tile([128, 512], F16, tag="et")
                                nc.scalar.activation(et[:, :nq], ps[:, :nq],
                                                     func=AF.Exp,
                                                     scale=SCALING)
                                if pend is not None:
                                    fin = flush_pend(pend)
                                    if fin is not None:
                                        head_fin[pend[0]] = fin
                                pend = (h, st, et, nq, off, rs_ps, pv_ps,
                                        st == nst - 1)
                        # h0 finalizes now (its rs chain ran during h1's
                        # scores); h1's last tile flushes behind h0's bc
                        # matmul, then finalizes after o_proj
                        finalize_head(cq, 0, *head_fin[0])
                        head_fin[1] = flush_pend(pend)
                        if cq > 0:
                            do_oproj(cq - 1)
                        finalize_head(cq, 1, *head_fin[1])
                    do_oproj(NCH - 1)
    nc.compile()
    return nc


_CACHE = {}


def _get(name):
    if name not in _CACHE:
        _CACHE[name] = _build_a() if name == "a" else _build_b()
    return _CACHE[name]


def _host_consts():
    # swap matrix S: (Sx)[2i] = -x[2i+1], (Sx)[2i+1] = x[2i]; we pass S^T,
    # block-diag over the two 64-row head slots.
    st64 = np.zeros((64, 64), dtype=np.float32)
    for i in range(32):
        st64[2 * i, 2 * i + 1] = 1.0
        st64[2 * i + 1, 2 * i] = -1.0
    swap2t = np.zeros((128, 128), dtype=np.float32)
    swap2t[0:64, 0:64] = st64
    swap2t[64:128, 64:128] = st64
    r = np.arange(128)
    # transposed-scores diagonal mask: S^T[ks, tq] valid iff tq >= ks.
    # -60000 is f16-safe and exp(scale*-60000) == 0.
    diagt = np.where(r[None, :] >= r[:, None], 0.0, -60000.0)
    diagt_t = np.ascontiguousarray(diagt.T).astype(np.float16)
    return st64, swap2t, diagt_t


def _rope_tables(positions):
    # duplicated-pair (interleaved) layout, rows stacked twice for 2 heads
    inv_freq = 1.0 / (THETA ** (np.arange(0, ROPE, 2, dtype=np.float32)
                                / ROPE))
    freqs = positions.astype(np.float32)[:, None] * inv_freq[None, :]  # [T,32]
    cos = np.cos(freqs).astype(np.float32)
    sin = np.sin(freqs).astype(np.float32)
    cos_dup = np.repeat(cos, 2, axis=1).T.copy()   # [64, T]
    sin_dup = np.repeat(sin, 2, axis=1).T.copy()
    cos2 = np.vstack([cos_dup, cos_dup])           # [128, T]
    sin2 = np.vstack([sin_dup, sin_dup])
    return (np.ascontiguousarray(cos_dup), np.ascontiguousarray(sin_dup),
            np.ascontiguousarray(cos2), np.ascontiguousarray(sin2))


def kernel(positions, hidden_states, w_fused, q_a_ln_w, kv_a_ln_w,
           w_qb, w_kvb, w_o):
    positions = np.asarray(positions)
    hidden_states = np.asarray(hidden_states, dtype=np.float32)
    w_fused = np.asarray(w_fused, dtype=np.float32)
    q_a_ln_w = np.asarray(q_a_ln_w, dtype=np.float32)
    kv_a_ln_w = np.asarray(kv_a_ln_w, dtype=np.float32)
    w_qb = np.asarray(w_qb, dtype=np.float32)
    w_kvb = np.asarray(w_kvb, dtype=np.float32)
    w_o = np.asarray(w_o, dtype=np.float32)

    st64, swap2t, diagt_t = _host_consts()
    swap2t16 = swap2t.astype(np.float16)
    st64_16 = st64.astype(np.float16)
    ident16 = np.eye(128, dtype=np.float16)
    w_fused16 = np.ascontiguousarray(w_fused.astype(np.float16))
    hidden_T16 = np.ascontiguousarray(hidden_states.T.astype(np.float16))
    cos_dup, sin_dup, cos2, sin2 = _rope_tables(positions)
    cos2 = cos2.astype(np.float16)
    sin2 = sin2.astype(np.float16)
    # fold LN weights into the b-projection rows (exact)
    w_qb_eff = w_qb * q_a_ln_w[:, None]
    w_kvb_eff = w_kvb * kv_a_ln_w[:, None]

    # ---- launch A: sequence-parallel fused projection + norms + k_pe rope
    nca = _get("a")
    in_a = []
    for c in range(NCORES):
        sl = slice(c * TS, (c + 1) * TS)
        in_a.append({
            "hidT_s": np.ascontiguousarray(hidden_T16[:, sl]),
            "w_fused": w_fused16,
            "cos_a": np.ascontiguousarray(cos_dup[:, sl]).astype(np.float16),
            "sin_a": np.ascontiguousarray(sin_dup[:, sl]).astype(np.float16),
            "swap64t": st64_16,
        })
    res_a = bass_utils.run_bass_kernel_spmd(nca, in_a,
                                            core_ids=list(range(NCORES)))
    q_aT = np.concatenate([res_a.results[c]["q_aT_s"]
                           for c in range(NCORES)], axis=1)
    kv_aT = np.concatenate([res_a.results[c]["kv_aT_s"]
                            for c in range(NCORES)], axis=1)
    k_peT = np.concatenate([res_a.results[c]["k_peT_s"]
                            for c in range(NCORES)], axis=1)

    # ---- launch B: head-parallel attention
    ncb = _get("b")
    in_b = []
    for c in range(NCORES):
        g0, g1 = 2 * c, 2 * c + 1
        wq_s = np.ascontiguousarray(
            w_qb_eff[:, g0 * (NOPE + ROPE):(g1 + 1) * (NOPE + ROPE)]
        ).astype(np.float16)
        wk = w_kvb_eff
        wkv_s = np.ascontiguousarray(np.concatenate([
            wk[:, g0 * 256:g0 * 256 + 128],        # h0 nope
            wk[:, g1 * 256:g1 * 256 + 128],        # h1 nope
            wk[:, g0 * 256 + 128:(g0 + 1) * 256],  # h0 v
            wk[:, g1 * 256 + 128:(g1 + 1) * 256],  # h1 v
        ], axis=1)).astype(np.float16)
        wo_s = np.ascontiguousarray(
            w_o[g0 * VDIM:(g1 + 1) * VDIM, :]).astype(np.float16)
        in_b.append({
            "q_aT": q_aT, "kv_aT": kv_aT, "k_peT": k_peT,
            "w_qb_s": wq_s, "w_kvb_s": wkv_s, "w_o_s": wo_s,
            "cos2": cos2, "sin2": sin2,
            "swap2t": swap2t16, "diagt_t": diagt_t, "ident": ident16,
        })
    res_b = bass_utils.run_bass_kernel_spmd(ncb, in_b,
                                            core_ids=list(range(NCORES)))
    out = res_b.results[0]["o_part"].astype(np.float64)
    for c in range(1, NCORES):
        out += res_b.results[c]["o_part"]
    return out.astype(np.float32)
